# revision 11
# baseline (speedup 1.0000x reference)
"""DocRE model kernel for 8 Trainium2 NeuronCores.

Data-parallel over the pair grid: core = b*4 + ib owns document b and
i-rows [8*ib, 8*ib+8) of the 32x32 entity-pair grid (256 pairs/core).
All weights are replicated; W_ext (49152x768, repacked partition-major
on the host) is streamed from HBM in 3MB chunks through fp16 matmuls
against group-bilinear tiles built on-chip.  The hs/ts factors are
round-tripped through DRAM so per-group partition-replicated layouts
(hsdup / tsd) can be produced by plain DMAs instead of PE broadcasts.
"""

import numpy as np

import concourse.bacc as bacc
import concourse.bass as bass
import concourse.tile as tile
from concourse import mybir
from concourse.bass_utils import run_bass_kernel_spmd
from concourse.masks import make_identity

F32 = mybir.dt.float32
F16 = mybir.dt.float16

B, L, H = 2, 1024, 768
E, M = 32, 4
EMB, BLK, NL = 768, 64, 97
G = EMB // BLK  # 12
LN_EPS = 1e-12

N_CORES = 8
IB = E // (N_CORES // B)     # 8 i-rows per core
NPAIR = IB * E               # 256 pairs per core
PT = NPAIR // 128            # 2 pair-tiles
KT = EMB * BLK // 128        # 384 k-tiles
WCH = 16                     # k-tiles per W_ext DMA chunk (3 MB each)
BKT = 4                      # k-tiles per DVE bilinear batch
CT = EMB // 128              # 6 feature chunks
KC = H // 128                # 6 contraction chunks of H
LC = L // 128                # 8 chunks of L
NENT = IB + E + 1            # 41 cols: [my 8 entities | all 32 | cls]
NE2 = NENT + 1


def _build_module():
    nc = bacc.Bacc("TRN2", target_bir_lowering=False, debug=False)

    seq_d = nc.dram_tensor("seq", [L, H], F16, kind="ExternalInput")
    S_d = nc.dram_tensor("S", [L, NENT], F16, kind="ExternalInput")
    Wh_d = nc.dram_tensor("Wh", [3 * H, EMB], F16, kind="ExternalInput")
    Wt_d = nc.dram_tensor("Wt", [3 * H, EMB], F16, kind="ExternalInput")
    bh_d = nc.dram_tensor("bh", [128, CT], F32, kind="ExternalInput")
    bt_d = nc.dram_tensor("bt", [128, CT], F32, kind="ExternalInput")
    # W_ext repacked host-side to partition-major [p, kt, o] so each chunk
    # DMA is one fully-contiguous ~24KB line per partition.
    Wx_d = nc.dram_tensor("Wx", [128, KT * EMB], F16, kind="ExternalInput")
    bx_d = nc.dram_tensor("bx", [128, EMB], F32, kind="ExternalInput")
    lng_d = nc.dram_tensor("lng", [128, EMB], F32, kind="ExternalInput")
    lnb_d = nc.dram_tensor("lnb", [128, EMB], F32, kind="ExternalInput")
    Wc_d = nc.dram_tensor("Wc", [EMB, NL], F32, kind="ExternalInput")
    out_d = nc.dram_tensor("out", [NPAIR, NL], F32, kind="ExternalOutput")

    with tile.TileContext(nc) as tc:
        with (
            tc.tile_pool(name="persist", bufs=1) as persist,
            tc.tile_pool(name="seqp", bufs=1) as seqp,
            tc.tile_pool(name="whp", bufs=9) as whp,
            tc.tile_pool(name="wxp", bufs=3) as wxp,
            tc.tile_pool(name="blp", bufs=4) as blp,
            tc.tile_pool(name="hsdupp", bufs=2) as hsdupp,
            tc.tile_pool(name="tsdp", bufs=2) as tsdp,
            tc.tile_pool(name="hstp", bufs=4) as hstp,
            tc.tile_pool(name="tmpp", bufs=3) as tmpp,
            tc.tile_pool(name="dramp", bufs=1, space="DRAM") as dramp,
            tc.tile_pool(name="psf", bufs=1, space="PSUM") as psf,
            tc.tile_pool(name="psg", bufs=2, space="PSUM") as psg,
        ):
            ident = persist.tile([128, 128], F32, name="ident")
            make_identity(nc, ident[:])

            # ---- per-column constants broadcast to all partitions ----
            bx_b = persist.tile([128, EMB], F32, name="bx_b")
            lng_b = persist.tile([128, EMB], F32, name="lng_b")
            lnb_b = persist.tile([128, EMB], F32, name="lnb_b")
            for tile_, src in ((bx_b, bx_d), (lng_b, lng_d), (lnb_b, lnb_d)):
                nc.sync.dma_start(tile_[:], src.ap())

            eps_t = persist.tile([128, 1], F32, name="eps")
            nc.vector.memset(eps_t[:], LN_EPS)

            # per-partition bias chunks bh/bt: [128, CT]
            bh_t = persist.tile([128, CT], F32, name="bh_t")
            bt_t = persist.tile([128, CT], F32, name="bt_t")
            for tile_, src in ((bh_t, bh_d), (bt_t, bt_d)):
                nc.sync.dma_start(tile_[:], src.ap())

            # ---- phase E: entity pooling  ent = S^T @ seq ----
            seq_t = seqp.tile([128, LC, H], F16, name="seq_t")
            S_t = seqp.tile([128, LC, NENT], F16, name="S_t")
            seq_re = seq_d.ap().rearrange("(c p) h -> p c h", p=128)
            S_re = S_d.ap().rearrange("(c p) n -> p c n", p=128)
            for kc in range(LC):
                nc.sync.dma_start(S_t[:, kc, :], S_re[:, kc, :])
                nc.sync.dma_start(seq_t[:, kc, :], seq_re[:, kc, :])

            ps_e0 = psg.tile([NENT, 512], F32, name="gen")
            ps_e1 = psg.tile([NENT, 256], F32, name="gen")
            for kc in range(LC):
                nc.tensor.matmul(ps_e0[:], S_t[:, kc, :], seq_t[:, kc, 0:512],
                                 start=(kc == 0), stop=(kc == LC - 1))
                nc.tensor.matmul(ps_e1[:], S_t[:, kc, :], seq_t[:, kc, 512:768],
                                 start=(kc == 0), stop=(kc == LC - 1))
            ent_nat = persist.tile([NENT, H], F32, name="ent_nat")
            nc.scalar.copy(ent_nat[:, 0:512], ps_e0[:])
            nc.scalar.copy(ent_nat[:, 512:768], ps_e1[:])

            # transpose ent -> entT [h, NENT]  (feeds the projection matmuls)
            entT = persist.tile([128, KC, NENT], F16, name="entT")
            for kc in range(KC):
                ps_tr = psg.tile([128, NENT], F32, name="gen")
                nc.tensor.transpose(ps_tr[:], ent_nat[:, kc * 128:(kc + 1) * 128],
                                    ident[:NENT, :NENT])
                nc.scalar.copy(entT[:, kc, :], ps_tr[:])

            # ---- phase A: A/B/C projections ----
            # natural layout first: X_nat = ent @ W_block  [41, 768], then
            # PE-transpose into ABCD[ct][:, m, :] ([c,41], m: Ah,Bh,At,Bt).
            ABCD = []
            for ct in range(CT):
                abcd_alloc = persist.tile([128, 4, NE2], F32, name=f"abcd{ct}")
                nc.vector.memset(abcd_alloc[:], 0.0)
                ABCD.append(abcd_alloc)

            # feat accumulators allocated early: the ts-side phase-A chains
            # borrow these idle PSUM banks so chains can run concurrently.
            ps_feat = [[psf.tile([128, 512], F32, name=f"pf{pt}a"),
                        psf.tile([128, 256], F32, name=f"pf{pt}b")]
                       for pt in range(PT)]

            def emit_ab_chain(m, w_d, blk, ps_pair=None):
                if ps_pair is None:
                    ps_n0 = psg.tile([NENT, 512], F32, name="gen")
                    ps_n1 = psg.tile([NENT, 256], F32, name="gen")
                else:
                    ps_n0 = ps_pair[0][:NENT, :]
                    ps_n1 = ps_pair[1][:NENT, :]
                for kc in range(KC):
                    w_t = whp.tile([128, EMB], F16, name="w_t")
                    nc.sync.dma_start(
                        w_t[:], w_d.ap()[blk * H + kc * 128: blk * H + (kc + 1) * 128, :])
                    nc.tensor.matmul(ps_n0[:], entT[:, kc, :], w_t[:, 0:512],
                                     start=(kc == 0), stop=(kc == KC - 1))
                    nc.tensor.matmul(ps_n1[:], entT[:, kc, :], w_t[:, 512:768],
                                     start=(kc == 0), stop=(kc == KC - 1))
                x_nat = tmpp.tile([NENT, EMB], F32, name="x_nat")
                nc.scalar.copy(x_nat[:, 0:512], ps_n0[:])
                nc.scalar.copy(x_nat[:, 512:768], ps_n1[:])
                for ct in range(CT):
                    ps_tr = psg.tile([128, NENT], F32, name="gen")
                    nc.tensor.transpose(ps_tr[:], x_nat[:, ct * 128:(ct + 1) * 128],
                                        ident[:NENT, :NENT])
                    nc.scalar.copy(ABCD[ct][:, m, 0:NENT], ps_tr[:])

            def emit_c_chain(m_sel, w_d, bias_t):
                ps_c0 = psg.tile([NENT, 512], F32, name="gen")
                ps_c1 = psg.tile([NENT, 256], F32, name="gen")
                for kc in range(KC):
                    w_t = whp.tile([128, EMB], F16, name="w_t")
                    nc.sync.dma_start(
                        w_t[:], w_d.ap()[2 * H + kc * 128: 2 * H + (kc + 1) * 128, :])
                    nc.tensor.matmul(ps_c0[:1, :], entT[:, kc, IB + E:IB + E + 1],
                                     w_t[:, 0:512],
                                     start=(kc == 0), stop=(kc == KC - 1))
                    nc.tensor.matmul(ps_c1[:1, :], entT[:, kc, IB + E:IB + E + 1],
                                     w_t[:, 512:768],
                                     start=(kc == 0), stop=(kc == KC - 1))
                c_nat = tmpp.tile([1, EMB], F32, name="c_nat")
                nc.scalar.copy(c_nat[:, 0:512], ps_c0[:1, :])
                nc.scalar.copy(c_nat[:, 512:768], ps_c1[:1, :])
                for ct in range(CT):
                    ps_tr = psg.tile([128, NENT], F32, name="gen")
                    nc.tensor.transpose(ps_tr[:, 0:1],
                                        c_nat[:, ct * 128:(ct + 1) * 128],
                                        ident[:1, :1])
                    nc.vector.tensor_tensor(ABCD[ct][:, m_sel, NENT:NENT + 1],
                                            ps_tr[:, 0:1],
                                            bias_t[:, ct:ct + 1],
                                            op=mybir.AluOpType.add)

            def colview(tile_, m, col0, ap_pat):
                return bass.AP(tensor=tile_.tensor,
                               offset=tile_.offset + m * NE2 + col0,
                               ap=[tile_.ap[0]] + ap_pat)

            # DRAM round-trip buffers for the pair factors.
            # ts_dram: natural feature order [768 rows, 256].
            # hs_dram: dup-friendly order: row (g*64 + par*32 + l) holds
            #   hs feature (g*64 + 2l + par)  -> per (g, par) block of 32
            #   rows is exactly what partitions [par*64,(par+1)*64) of a
            #   group's hsdup tile need, contiguously.
            ts_dram = dramp.tile([EMB, 256], F16, name="ts_dram")
            hs_dram = dramp.tile([EMB, 256], F16, name="hs_dram")

            def emit_tanh(ct, ma, mb, cola, colb, msel, dst_dram, dup_order):
                # x[p, il, j] = A[p, cola+?] + B[p, ?]  (see colview patterns)
                tmp = tmpp.tile([128, 8, 32], F32, name="tmp")
                nc.vector.tensor_tensor(
                    tmp[:], colview(ABCD[ct], ma, cola[0], cola[1]),
                    colview(ABCD[ct], mb, colb[0], colb[1]),
                    op=mybir.AluOpType.add)
                xt = hstp.tile([128, 256], F16, name="xt")
                nc.scalar.activation(
                    xt[:].rearrange("p (a b) -> p a b", a=8),
                    tmp[:], mybir.ActivationFunctionType.Tanh,
                    bias=ABCD[ct][:, msel, NENT:NENT + 1], scale=1.0)
                if dup_order:
                    # partition p = ph*64 + pl*2 + par  (feature ct*128+p)
                    # -> row g*64 + par*32 + pl with g = 2ct + ph; one DMA
                    # per ph half keeps the dst AP at 3 dims.
                    for ph in range(2):
                        dst = bass.AP(
                            tensor=dst_dram.tensor,
                            offset=dst_dram.offset + (ct * 128 + ph * 64) * 256,
                            ap=[[256, 32], [32 * 256, 2], [1, 256]])
                        nc.scalar.dma_start(dst, xt[ph * 64:(ph + 1) * 64, :])
                else:
                    nc.scalar.dma_start(dst_dram[ct * 128:(ct + 1) * 128, :], xt[:])

            # ---- ts side: projections, tanh, natural-order store ----
            emit_c_chain(3, Wt_d, bt_t)
            emit_ab_chain(2, Wt_d, 0, ps_feat[0])
            emit_ab_chain(3, Wt_d, 1, ps_feat[1])
            for ct in range(CT):
                # ts[pair=(il,j)] = tanh(At[j] + Bt[il] + Ct + bt)
                emit_tanh(ct, 2, 3, (IB, [[0, 8], [1, 32]]), (0, [[1, 8], [0, 32]]),
                          3, ts_dram, dup_order=False)

            # ---- hs side ----
            emit_c_chain(0, Wh_d, bh_t)
            emit_ab_chain(0, Wh_d, 0, ps_feat[0])
            emit_ab_chain(1, Wh_d, 1, ps_feat[1])
            for ct in range(CT):
                # hs[pair=(il,j)] = tanh(Ah[il] + Bh[j] + Ch + bh)
                emit_tanh(ct, 0, 1, (0, [[1, 8], [0, 32]]), (IB, [[0, 8], [1, 32]]),
                          0, hs_dram, dup_order=True)

            # ---- phase M: main contraction over W_ext ----
            wx_ch = None
            for g in range(G):
                # per-group factor tiles via DMA (no PE broadcast):
                # tsd[p, pair]      = ts[g*64 + p%64, pair]
                # hsdup[p, l, pair] = hs[g*64 + 2l + p//64, pair]
                tsd = tsdp.tile([128, 256], F16, name="tsd")
                for ph in range(2):
                    nc.sync.dma_start(tsd[ph * 64:(ph + 1) * 64, :],
                                      ts_dram[g * 64:(g + 1) * 64, :])
                hsdup = hsdupp.tile([128, 32, 256], F16, name="hsdup")
                for ph in range(2):
                    src = bass.AP(
                        tensor=hs_dram.tensor,
                        offset=hs_dram.offset + (g * 64 + ph * 32) * 256,
                        ap=[[0, 64], [1, 32 * 256]])
                    nc.gpsimd.dma_start(
                        hsdup[ph * 64:(ph + 1) * 64, :, :].rearrange(
                            "p l c -> p (l c)"), src)

                for bb in range(32 // BKT):
                    kt0 = g * 32 + bb * BKT
                    if kt0 % WCH == 0:
                        wx_ch = wxp.tile([128, WCH * EMB], F16, name="wx_ch")
                        c0 = kt0 * EMB
                        nc.sync.dma_start(wx_ch[:], Wx_d.ap()[:, c0:c0 + WCH * EMB])

                    bl4 = blp.tile([128, BKT, 256], F16, name="bl4")
                    tsd_b = bass.AP(tensor=tsd.tensor, offset=tsd.offset,
                                    ap=[tsd.ap[0], [0, BKT], [1, 256]])
                    nc.vector.tensor_tensor(bl4[:], hsdup[:, bb * BKT:(bb + 1) * BKT, :],
                                            tsd_b, op=mybir.AluOpType.mult)
                    for q in range(BKT):
                        kt = kt0 + q
                        kl = kt % WCH
                        for pt in range(PT):
                            lhsT = bl4[:, q, pt * 128:(pt + 1) * 128]
                            nc.tensor.matmul(
                                ps_feat[pt][0][:], lhsT,
                                wx_ch[:, kl * EMB:kl * EMB + 512],
                                start=(kt == 0), stop=(kt == KT - 1))
                            nc.tensor.matmul(
                                ps_feat[pt][1][:], lhsT,
                                wx_ch[:, kl * EMB + 512:(kl + 1) * EMB],
                                start=(kt == 0), stop=(kt == KT - 1))

            # ---- phase L: bias, relu, layernorm, classifier ----
            wc_t = persist.tile([128, CT, NL], F32, name="wc_t")
            nc.sync.dma_start(wc_t[:], Wc_d.ap().rearrange("(c p) n -> p c n", p=128))

            for pt in range(PT):
                feat = persist.tile([128, EMB], F32, name=f"feat{pt}")
                nc.vector.tensor_tensor(feat[:, 0:512], ps_feat[pt][0][:],
                                        bx_b[:, 0:512], op=mybir.AluOpType.add)
                nc.vector.tensor_tensor(feat[:, 512:768], ps_feat[pt][1][:],
                                        bx_b[:, 512:768], op=mybir.AluOpType.add)
                nc.scalar.activation(feat[:], feat[:],
                                     mybir.ActivationFunctionType.Relu,
                                     bias=0.0, scale=1.0)

                stats = tmpp.tile([128, 3, 6], F32, name="stats")
                f_re = feat.rearrange("p (c f) -> p c f", c=3)
                for c in range(3):
                    nc.vector.bn_stats(stats[:, c, :], f_re[:, c, :])
                mv = tmpp.tile([128, 2], F32, name="mv")
                nc.vector.bn_aggr(mv[:], stats[:])
                sd = tmpp.tile([128, 1], F32, name="sd")
                nc.scalar.activation(sd[:], mv[:, 1:2],
                                     mybir.ActivationFunctionType.Sqrt,
                                     bias=eps_t[:], scale=1.0)
                rstd = tmpp.tile([128, 1], F32, name="rstd")
                nc.vector.reciprocal(rstd[:], sd[:])

                ln = persist.tile([128, EMB], F32, name=f"ln{pt}")
                nc.vector.tensor_scalar(ln[:], feat[:], mv[:, 0:1], rstd[:],
                                        op0=mybir.AluOpType.subtract,
                                        op1=mybir.AluOpType.mult)
                nc.vector.tensor_tensor(ln[:], ln[:], lng_b[:],
                                        op=mybir.AluOpType.mult)
                nc.vector.tensor_tensor(ln[:], ln[:], lnb_b[:],
                                        op=mybir.AluOpType.add)

                lnT = persist.tile([128, CT, 128], F32, name=f"lnT{pt}")
                for ct in range(CT):
                    ps_tr2 = psg.tile([128, 128], F32, name="gen")
                    nc.tensor.transpose(ps_tr2[:], ln[:, ct * 128:(ct + 1) * 128],
                                        ident[:])
                    nc.scalar.copy(lnT[:, ct, :], ps_tr2[:])

                ps_lg = psg.tile([128, NL], F32, name="gen")
                for ct in range(CT):
                    nc.tensor.matmul(ps_lg[:], lnT[:, ct, :], wc_t[:, ct, :],
                                     start=(ct == 0), stop=(ct == CT - 1))
                out_sb = tmpp.tile([128, NL], F32, name="out_sb")
                nc.scalar.copy(out_sb[:], ps_lg[:])
                nc.scalar.dma_start(out_d.ap()[pt * 128:(pt + 1) * 128, :], out_sb[:])

    nc.compile()
    return nc


_NC_CACHE = []


def _get_module():
    if not _NC_CACHE:
        _NC_CACHE.append(_build_module())
    return _NC_CACHE[0]


def _build_inputs(seq, starts, ends, mention_mask, W_head, b_head, W_tail, b_tail,
                  W_ext, b_ext, ln_g, ln_b, W_cls):
    seq = np.asarray(seq, np.float32)
    starts = np.asarray(starts, np.int64)
    ends = np.asarray(ends, np.int64)
    mask = np.asarray(mention_mask, np.float32)

    # per-document entity selection matrix: ent = Sb^T @ seq[b]
    S_b = np.zeros((B, L, E), np.float32)
    denom = np.maximum(mask.sum(axis=2), 1.0)          # [B, E]
    w = mask * 0.5 / denom[:, :, None]                 # [B, E, M]
    for b in range(B):
        for e in range(E):
            np.add.at(S_b[b, :, e], starts[b, e] + 1, w[b, e])
            np.add.at(S_b[b, :, e], ends[b, e], w[b, e])

    cls_col = np.zeros((L, 1), np.float32)
    cls_col[0, 0] = 1.0

    shared = {
        "Wh": np.ascontiguousarray(np.asarray(W_head, np.float32).astype(np.float16)),
        "Wt": np.ascontiguousarray(np.asarray(W_tail, np.float32).astype(np.float16)),
        "bh": np.ascontiguousarray(np.asarray(b_head, np.float32).reshape(CT, 128).T),
        "bt": np.ascontiguousarray(np.asarray(b_tail, np.float32).reshape(CT, 128).T),
        "Wx": np.ascontiguousarray(
            np.asarray(W_ext).astype(np.float16)
            .reshape(KT, 128, EMB).transpose(1, 0, 2).reshape(128, KT * EMB)),
        "bx": np.ascontiguousarray(np.broadcast_to(np.asarray(b_ext, np.float32), (128, EMB))),
        "lng": np.ascontiguousarray(np.broadcast_to(np.asarray(ln_g, np.float32), (128, EMB))),
        "lnb": np.ascontiguousarray(np.broadcast_to(np.asarray(ln_b, np.float32), (128, EMB))),
        "Wc": np.ascontiguousarray(W_cls, dtype=np.float32),
    }
    in_maps = []
    for core in range(N_CORES):
        b, ib = core // 4, core % 4
        S_core = np.concatenate(
            [S_b[b][:, ib * IB:(ib + 1) * IB], S_b[b], cls_col], axis=1)
        in_maps.append({
            "seq": np.ascontiguousarray(seq[b].astype(np.float16)),
            "S": np.ascontiguousarray(S_core.astype(np.float16)),
            **shared,
        })
    return in_maps


def kernel(**inputs) -> np.ndarray:
    nc = _get_module()
    in_maps = _build_inputs(**inputs)
    res = run_bass_kernel_spmd(nc, in_maps, core_ids=list(range(N_CORES)))
    outs = np.stack([res.results[c]["out"] for c in range(N_CORES)])  # [8,256,97]
    return outs.reshape(B, 4, IB, E, NL).reshape(B, E, E, NL)


# revision 15
# speedup vs baseline: 1.1088x; 1.1088x over previous
"""DocRE model kernel for 8 Trainium2 NeuronCores.

Data-parallel over the pair grid: core = b*4 + ib owns document b and
i-rows [8*ib, 8*ib+8) of the 32x32 entity-pair grid (256 pairs/core).
All weights are replicated; W_ext (49152x768, repacked partition-major
on the host) is streamed from HBM in 3MB chunks through fp16 matmuls
against group-bilinear tiles built on-chip.  The hs/ts factors are
round-tripped through DRAM so per-group partition-replicated layouts
(hsdup / tsd) can be produced by plain DMAs instead of PE broadcasts.
"""

import numpy as np

import concourse.bacc as bacc
import concourse.bass as bass
import concourse.tile as tile
from concourse import mybir
from concourse.bass_utils import run_bass_kernel_spmd
from concourse.masks import make_identity

F32 = mybir.dt.float32
F16 = mybir.dt.float16

B, L, H = 2, 1024, 768
E, M = 32, 4
EMB, BLK, NL = 768, 64, 97
G = EMB // BLK  # 12
LN_EPS = 1e-12

N_CORES = 8
IB = E // (N_CORES // B)     # 8 i-rows per core
NPAIR = IB * E               # 256 pairs per core
PT = NPAIR // 128            # 2 pair-tiles
KT = EMB * BLK // 128        # 384 k-tiles
WCH = 16                     # k-tiles per W_ext DMA chunk (3 MB each)
BKT = 4                      # k-tiles per DVE bilinear batch
CT = EMB // 128              # 6 feature chunks
KC = H // 128                # 6 contraction chunks of H
LC = L // 128                # 8 chunks of L
NENT = IB + E + 1            # 41 cols: [my 8 entities | all 32 | cls]
NE2 = NENT + 1


def _build_module():
    nc = bacc.Bacc("TRN2", target_bir_lowering=False, debug=False)

    seq_d = nc.dram_tensor("seq", [L, H], F16, kind="ExternalInput")
    S_d = nc.dram_tensor("S", [L, NENT], F16, kind="ExternalInput")
    Wh_d = nc.dram_tensor("Wh", [3 * H, EMB], F16, kind="ExternalInput")
    Wt_d = nc.dram_tensor("Wt", [3 * H, EMB], F16, kind="ExternalInput")
    bh_d = nc.dram_tensor("bh", [128, CT], F32, kind="ExternalInput")
    bt_d = nc.dram_tensor("bt", [128, CT], F32, kind="ExternalInput")
    # W_ext repacked host-side to partition-major [p, kt, o] so each chunk
    # DMA is one fully-contiguous ~24KB line per partition.
    Wx_d = nc.dram_tensor("Wx", [128, KT * EMB], F16, kind="ExternalInput")
    bx_d = nc.dram_tensor("bx", [128, EMB], F32, kind="ExternalInput")
    lng_d = nc.dram_tensor("lng", [128, EMB], F32, kind="ExternalInput")
    lnb_d = nc.dram_tensor("lnb", [128, EMB], F32, kind="ExternalInput")
    Wc_d = nc.dram_tensor("Wc", [EMB, NL], F32, kind="ExternalInput")
    out_d = nc.dram_tensor("out", [NPAIR, NL], F32, kind="ExternalOutput")

    with tile.TileContext(nc) as tc:
        with (
            tc.tile_pool(name="persist", bufs=1) as persist,
            tc.tile_pool(name="seqp", bufs=1) as seqp,
            tc.tile_pool(name="whp", bufs=9) as whp,
            tc.tile_pool(name="wxp", bufs=3) as wxp,
            tc.tile_pool(name="blp", bufs=4) as blp,
            tc.tile_pool(name="hsdupp", bufs=2) as hsdupp,
            tc.tile_pool(name="tsdp", bufs=2) as tsdp,
            tc.tile_pool(name="hstp", bufs=4) as hstp,
            tc.tile_pool(name="tmpp", bufs=3) as tmpp,
            tc.tile_pool(name="dramp", bufs=1, space="DRAM") as dramp,
            tc.tile_pool(name="psf", bufs=1, space="PSUM") as psf,
            tc.tile_pool(name="psg", bufs=2, space="PSUM") as psg,
        ):
            ident = persist.tile([128, 128], F32, name="ident")
            make_identity(nc, ident[:])

            # ---- per-column constants broadcast to all partitions ----
            bx_b = persist.tile([128, EMB], F32, name="bx_b")
            lng_b = persist.tile([128, EMB], F32, name="lng_b")
            lnb_b = persist.tile([128, EMB], F32, name="lnb_b")
            for tile_, src in ((bx_b, bx_d), (lng_b, lng_d), (lnb_b, lnb_d)):
                nc.sync.dma_start(tile_[:], src.ap())

            eps_t = persist.tile([128, 1], F32, name="eps")
            nc.vector.memset(eps_t[:], LN_EPS)

            # per-partition bias chunks bh/bt: [128, CT]
            bh_t = persist.tile([128, CT], F32, name="bh_t")
            bt_t = persist.tile([128, CT], F32, name="bt_t")
            for tile_, src in ((bh_t, bh_d), (bt_t, bt_d)):
                nc.sync.dma_start(tile_[:], src.ap())

            # ---- phase E: entity pooling  ent = S^T @ seq ----
            seq_t = seqp.tile([128, LC, H], F16, name="seq_t")
            S_t = seqp.tile([128, LC, NENT], F16, name="S_t")
            seq_re = seq_d.ap().rearrange("(c p) h -> p c h", p=128)
            S_re = S_d.ap().rearrange("(c p) n -> p c n", p=128)
            for kc in range(LC):
                nc.sync.dma_start(S_t[:, kc, :], S_re[:, kc, :])
                nc.sync.dma_start(seq_t[:, kc, :], seq_re[:, kc, :])

            ps_e0 = psg.tile([NENT, 512], F32, name="gen")
            ps_e1 = psg.tile([NENT, 256], F32, name="gen")
            for kc in range(LC):
                nc.tensor.matmul(ps_e0[:], S_t[:, kc, :], seq_t[:, kc, 0:512],
                                 start=(kc == 0), stop=(kc == LC - 1))
                nc.tensor.matmul(ps_e1[:], S_t[:, kc, :], seq_t[:, kc, 512:768],
                                 start=(kc == 0), stop=(kc == LC - 1))
            ent_nat = persist.tile([NENT, H], F32, name="ent_nat")
            nc.scalar.copy(ent_nat[:, 0:512], ps_e0[:])
            nc.scalar.copy(ent_nat[:, 512:768], ps_e1[:])

            # transpose ent -> entT [h, NENT]  (feeds the projection matmuls)
            entT = persist.tile([128, KC, NENT], F16, name="entT")
            for kc in range(KC):
                ps_tr = psg.tile([128, NENT], F32, name="gen")
                nc.tensor.transpose(ps_tr[:], ent_nat[:, kc * 128:(kc + 1) * 128],
                                    ident[:NENT, :NENT])
                nc.scalar.copy(entT[:, kc, :], ps_tr[:])

            # ---- phase A: A/B/C projections ----
            # natural layout first: X_nat = ent @ W_block  [41, 768], then
            # PE-transpose into ABCD[ct][:, m, :] ([c,41], m: Ah,Bh,At,Bt).
            ABCD = []
            for ct in range(CT):
                abcd_alloc = persist.tile([128, 4, NE2], F32, name=f"abcd{ct}")
                nc.vector.memset(abcd_alloc[:], 0.0)
                ABCD.append(abcd_alloc)

            # feat accumulators allocated early: the ts-side phase-A chains
            # borrow these idle PSUM banks so chains can run concurrently.
            ps_feat = [[psf.tile([128, 512], F32, name=f"pf{pt}a"),
                        psf.tile([128, 256], F32, name=f"pf{pt}b")]
                       for pt in range(PT)]

            def emit_ab_chain(m, w_d, blk, ps_pair=None):
                if ps_pair is None:
                    ps_n0 = psg.tile([NENT, 512], F32, name="gen")
                    ps_n1 = psg.tile([NENT, 256], F32, name="gen")
                else:
                    ps_n0 = ps_pair[0][:NENT, :]
                    ps_n1 = ps_pair[1][:NENT, :]
                for kc in range(KC):
                    w_t = whp.tile([128, EMB], F16, name="w_t")
                    nc.sync.dma_start(
                        w_t[:], w_d.ap()[blk * H + kc * 128: blk * H + (kc + 1) * 128, :])
                    nc.tensor.matmul(ps_n0[:], entT[:, kc, :], w_t[:, 0:512],
                                     start=(kc == 0), stop=(kc == KC - 1))
                    nc.tensor.matmul(ps_n1[:], entT[:, kc, :], w_t[:, 512:768],
                                     start=(kc == 0), stop=(kc == KC - 1))
                x_nat = tmpp.tile([NENT, EMB], F32, name="x_nat")
                nc.scalar.copy(x_nat[:, 0:512], ps_n0[:])
                nc.scalar.copy(x_nat[:, 512:768], ps_n1[:])
                for ct in range(CT):
                    ps_tr = psg.tile([128, NENT], F32, name="gen")
                    nc.tensor.transpose(ps_tr[:], x_nat[:, ct * 128:(ct + 1) * 128],
                                        ident[:NENT, :NENT])
                    nc.scalar.copy(ABCD[ct][:, m, 0:NENT], ps_tr[:])

            def emit_c_chain(m_sel, w_d, bias_t):
                ps_c0 = psg.tile([NENT, 512], F32, name="gen")
                ps_c1 = psg.tile([NENT, 256], F32, name="gen")
                for kc in range(KC):
                    w_t = whp.tile([128, EMB], F16, name="w_t")
                    nc.sync.dma_start(
                        w_t[:], w_d.ap()[2 * H + kc * 128: 2 * H + (kc + 1) * 128, :])
                    nc.tensor.matmul(ps_c0[:1, :], entT[:, kc, IB + E:IB + E + 1],
                                     w_t[:, 0:512],
                                     start=(kc == 0), stop=(kc == KC - 1))
                    nc.tensor.matmul(ps_c1[:1, :], entT[:, kc, IB + E:IB + E + 1],
                                     w_t[:, 512:768],
                                     start=(kc == 0), stop=(kc == KC - 1))
                c_nat = tmpp.tile([1, EMB], F32, name="c_nat")
                nc.scalar.copy(c_nat[:, 0:512], ps_c0[:1, :])
                nc.scalar.copy(c_nat[:, 512:768], ps_c1[:1, :])
                for ct in range(CT):
                    ps_tr = psg.tile([128, NENT], F32, name="gen")
                    nc.tensor.transpose(ps_tr[:, 0:1],
                                        c_nat[:, ct * 128:(ct + 1) * 128],
                                        ident[:1, :1])
                    nc.vector.tensor_tensor(ABCD[ct][:, m_sel, NENT:NENT + 1],
                                            ps_tr[:, 0:1],
                                            bias_t[:, ct:ct + 1],
                                            op=mybir.AluOpType.add)

            def colview(tile_, m, col0, ap_pat):
                return bass.AP(tensor=tile_.tensor,
                               offset=tile_.offset + m * NE2 + col0,
                               ap=[tile_.ap[0]] + ap_pat)

            # DRAM round-trip buffers for the pair factors.  k-tiles tile a
            # group's 64x64 (i,j) grid as [8 di x 16 pj] so hs replicates
            # 16x and ts 8x (24x total vs 66x for [2 x 64]).
            # ts_dram: natural feature order [768 rows, 256].
            # hs_dram: digit-swapped order: row (g*64 + di*8 + ib2) holds
            #   hs feature (g*64 + ib2*8 + di) -> the 8 rows a (g, di)
            #   read needs are one contiguous 4KB block.
            ts_dram = dramp.tile([EMB, 256], F16, name="ts_dram")
            hs_dram = dramp.tile([EMB, 256], F16, name="hs_dram")

            def emit_tanh(ct, ma, mb, cola, colb, msel, dst_dram, dup_order):
                # x[p, il, j] = A[p, cola+?] + B[p, ?]  (see colview patterns)
                tmp = tmpp.tile([128, 8, 32], F32, name="tmp")
                nc.vector.tensor_tensor(
                    tmp[:], colview(ABCD[ct], ma, cola[0], cola[1]),
                    colview(ABCD[ct], mb, colb[0], colb[1]),
                    op=mybir.AluOpType.add)
                xt = hstp.tile([128, 256], F16, name="xt")
                nc.scalar.activation(
                    xt[:].rearrange("p (a b) -> p a b", a=8),
                    tmp[:], mybir.ActivationFunctionType.Tanh,
                    bias=ABCD[ct][:, msel, NENT:NENT + 1], scale=1.0)
                if dup_order:
                    # partition p = ph*64 + ib2*8 + di (feature ct*128+p)
                    # -> row g*64 + di*8 + ib2 with g = 2ct + ph; one DMA
                    # per ph half keeps the dst AP at 3 dims.
                    for ph in range(2):
                        dst = bass.AP(
                            tensor=dst_dram.tensor,
                            offset=dst_dram.offset + (ct * 128 + ph * 64) * 256,
                            ap=[[256, 8], [8 * 256, 8], [1, 256]])
                        nc.scalar.dma_start(dst, xt[ph * 64:(ph + 1) * 64, :])
                else:
                    nc.scalar.dma_start(dst_dram[ct * 128:(ct + 1) * 128, :], xt[:])

            # ---- ts side: projections, tanh, natural-order store ----
            emit_c_chain(3, Wt_d, bt_t)
            emit_ab_chain(2, Wt_d, 0, ps_feat[0])
            emit_ab_chain(3, Wt_d, 1, ps_feat[1])
            for ct in range(CT):
                # ts[pair=(il,j)] = tanh(At[j] + Bt[il] + Ct + bt)
                emit_tanh(ct, 2, 3, (IB, [[0, 8], [1, 32]]), (0, [[1, 8], [0, 32]]),
                          3, ts_dram, dup_order=False)

            # ---- hs side ----
            emit_c_chain(0, Wh_d, bh_t)
            emit_ab_chain(0, Wh_d, 0, ps_feat[0])
            emit_ab_chain(1, Wh_d, 1, ps_feat[1])
            for ct in range(CT):
                # hs[pair=(il,j)] = tanh(Ah[il] + Bh[j] + Ch + bh)
                emit_tanh(ct, 0, 1, (0, [[1, 8], [0, 32]]), (IB, [[0, 8], [1, 32]]),
                          0, hs_dram, dup_order=True)

            # ---- phase M: main contraction over W_ext ----
            # k-tile (g, ib2, jb): partition p = di*16 + pj covers k-row
            # g*4096 + (ib2*8+di)*64 + jb*16 + pj.  Factor tiles per group:
            # hsdup[p, ib2, pair] = hs[g*64 + ib2*8 + p//16, pair]
            # tsdup[p, jb, pair]  = ts[g*64 + jb*16 + p%16, pair]
            wx_ch = None
            for g in range(G):
                hsdup = hsdupp.tile([128, 8, 256], F16, name="hsdup")
                for di in range(8):
                    src = bass.AP(
                        tensor=hs_dram.tensor,
                        offset=hs_dram.offset + (g * 64 + di * 8) * 256,
                        ap=[[0, 16], [1, 8 * 256]])
                    nc.sync.dma_start(
                        hsdup[di * 16:(di + 1) * 16, :, :].rearrange(
                            "p l c -> p (l c)"), src)
                tsdup = tsdp.tile([128, 4, 256], F16, name="tsdup")
                for di in range(8):
                    src = bass.AP(
                        tensor=ts_dram.tensor,
                        offset=ts_dram.offset + g * 64 * 256,
                        ap=[[256, 16], [16 * 256, 4], [1, 256]])
                    nc.sync.dma_start(tsdup[di * 16:(di + 1) * 16, :, :], src)

                for ib2 in range(8):
                    kt0 = g * 32 + ib2 * 4
                    if kt0 % WCH == 0:
                        wx_ch = wxp.tile([128, WCH * EMB], F16, name="wx_ch")
                        c0 = kt0 * EMB
                        nc.sync.dma_start(wx_ch[:], Wx_d.ap()[:, c0:c0 + WCH * EMB])

                    bl4 = blp.tile([128, 4, 256], F16, name="bl4")
                    hs_b = bass.AP(tensor=hsdup.tensor,
                                   offset=hsdup.offset + ib2 * 256,
                                   ap=[hsdup.ap[0], [0, 4], [1, 256]])
                    nc.vector.tensor_tensor(bl4[:], hs_b, tsdup[:],
                                            op=mybir.AluOpType.mult)
                    for jb in range(4):
                        kt = kt0 + jb
                        kl = kt % WCH
                        for pt in range(PT):
                            lhsT = bl4[:, jb, pt * 128:(pt + 1) * 128]
                            nc.tensor.matmul(
                                ps_feat[pt][0][:], lhsT,
                                wx_ch[:, kl * EMB:kl * EMB + 512],
                                start=(kt == 0), stop=(kt == KT - 1))
                            nc.tensor.matmul(
                                ps_feat[pt][1][:], lhsT,
                                wx_ch[:, kl * EMB + 512:(kl + 1) * EMB],
                                start=(kt == 0), stop=(kt == KT - 1))

            # ---- phase L: bias, relu, layernorm, classifier ----
            wc_t = persist.tile([128, CT, NL], F32, name="wc_t")
            nc.sync.dma_start(wc_t[:], Wc_d.ap().rearrange("(c p) n -> p c n", p=128))

            for pt in range(PT):
                feat = persist.tile([128, EMB], F32, name=f"feat{pt}")
                nc.vector.tensor_tensor(feat[:, 0:512], ps_feat[pt][0][:],
                                        bx_b[:, 0:512], op=mybir.AluOpType.add)
                nc.vector.tensor_tensor(feat[:, 512:768], ps_feat[pt][1][:],
                                        bx_b[:, 512:768], op=mybir.AluOpType.add)
                nc.scalar.activation(feat[:], feat[:],
                                     mybir.ActivationFunctionType.Relu,
                                     bias=0.0, scale=1.0)

                stats = tmpp.tile([128, 3, 6], F32, name="stats")
                f_re = feat.rearrange("p (c f) -> p c f", c=3)
                for c in range(3):
                    nc.vector.bn_stats(stats[:, c, :], f_re[:, c, :])
                mv = tmpp.tile([128, 2], F32, name="mv")
                nc.vector.bn_aggr(mv[:], stats[:])
                sd = tmpp.tile([128, 1], F32, name="sd")
                nc.scalar.activation(sd[:], mv[:, 1:2],
                                     mybir.ActivationFunctionType.Sqrt,
                                     bias=eps_t[:], scale=1.0)
                rstd = tmpp.tile([128, 1], F32, name="rstd")
                nc.vector.reciprocal(rstd[:], sd[:])

                ln = persist.tile([128, EMB], F32, name=f"ln{pt}")
                nc.vector.tensor_scalar(ln[:], feat[:], mv[:, 0:1], rstd[:],
                                        op0=mybir.AluOpType.subtract,
                                        op1=mybir.AluOpType.mult)
                nc.vector.tensor_tensor(ln[:], ln[:], lng_b[:],
                                        op=mybir.AluOpType.mult)
                nc.vector.tensor_tensor(ln[:], ln[:], lnb_b[:],
                                        op=mybir.AluOpType.add)

                lnT = persist.tile([128, CT, 128], F32, name=f"lnT{pt}")
                for ct in range(CT):
                    ps_tr2 = psg.tile([128, 128], F32, name="gen")
                    nc.tensor.transpose(ps_tr2[:], ln[:, ct * 128:(ct + 1) * 128],
                                        ident[:])
                    nc.scalar.copy(lnT[:, ct, :], ps_tr2[:])

                ps_lg = psg.tile([128, NL], F32, name="gen")
                for ct in range(CT):
                    nc.tensor.matmul(ps_lg[:], lnT[:, ct, :], wc_t[:, ct, :],
                                     start=(ct == 0), stop=(ct == CT - 1))
                out_sb = tmpp.tile([128, NL], F32, name="out_sb")
                nc.scalar.copy(out_sb[:], ps_lg[:])
                nc.scalar.dma_start(out_d.ap()[pt * 128:(pt + 1) * 128, :], out_sb[:])

    nc.compile()
    return nc


_NC_CACHE = []


def _get_module():
    if not _NC_CACHE:
        _NC_CACHE.append(_build_module())
    return _NC_CACHE[0]


def _build_inputs(seq, starts, ends, mention_mask, W_head, b_head, W_tail, b_tail,
                  W_ext, b_ext, ln_g, ln_b, W_cls):
    seq = np.asarray(seq, np.float32)
    starts = np.asarray(starts, np.int64)
    ends = np.asarray(ends, np.int64)
    mask = np.asarray(mention_mask, np.float32)

    # per-document entity selection matrix: ent = Sb^T @ seq[b]
    S_b = np.zeros((B, L, E), np.float32)
    denom = np.maximum(mask.sum(axis=2), 1.0)          # [B, E]
    w = mask * 0.5 / denom[:, :, None]                 # [B, E, M]
    for b in range(B):
        for e in range(E):
            np.add.at(S_b[b, :, e], starts[b, e] + 1, w[b, e])
            np.add.at(S_b[b, :, e], ends[b, e], w[b, e])

    cls_col = np.zeros((L, 1), np.float32)
    cls_col[0, 0] = 1.0

    shared = {
        "Wh": np.ascontiguousarray(np.asarray(W_head, np.float32).astype(np.float16)),
        "Wt": np.ascontiguousarray(np.asarray(W_tail, np.float32).astype(np.float16)),
        "bh": np.ascontiguousarray(np.asarray(b_head, np.float32).reshape(CT, 128).T),
        "bt": np.ascontiguousarray(np.asarray(b_tail, np.float32).reshape(CT, 128).T),
        # partition p = di*16+pj, kt = (g, ib2, jb); row k = g*4096 +
        # (ib2*8+di)*64 + jb*16 + pj
        "Wx": np.ascontiguousarray(
            np.asarray(W_ext).astype(np.float16)
            .reshape(G, 8, 8, 4, 16, EMB).transpose(2, 4, 0, 1, 3, 5)
            .reshape(128, KT * EMB)),
        "bx": np.ascontiguousarray(np.broadcast_to(np.asarray(b_ext, np.float32), (128, EMB))),
        "lng": np.ascontiguousarray(np.broadcast_to(np.asarray(ln_g, np.float32), (128, EMB))),
        "lnb": np.ascontiguousarray(np.broadcast_to(np.asarray(ln_b, np.float32), (128, EMB))),
        "Wc": np.ascontiguousarray(W_cls, dtype=np.float32),
    }
    in_maps = []
    for core in range(N_CORES):
        b, ib = core // 4, core % 4
        S_core = np.concatenate(
            [S_b[b][:, ib * IB:(ib + 1) * IB], S_b[b], cls_col], axis=1)
        in_maps.append({
            "seq": np.ascontiguousarray(seq[b].astype(np.float16)),
            "S": np.ascontiguousarray(S_core.astype(np.float16)),
            **shared,
        })
    return in_maps


def kernel(**inputs) -> np.ndarray:
    nc = _get_module()
    in_maps = _build_inputs(**inputs)
    res = run_bass_kernel_spmd(nc, in_maps, core_ids=list(range(N_CORES)))
    outs = np.stack([res.results[c]["out"] for c in range(N_CORES)])  # [8,256,97]
    return outs.reshape(B, 4, IB, E, NL).reshape(B, E, E, NL)


# revision 27
# speedup vs baseline: 1.1727x; 1.0576x over previous
"""DocRE model kernel for 8 Trainium2 NeuronCores.

Data-parallel over the pair grid: core = b*4 + ib owns document b and
i-rows [8*ib, 8*ib+8) of the 32x32 entity-pair grid (256 pairs/core).
All weights are replicated; W_ext (49152x768, repacked partition-major
on the host) is streamed from HBM in 3MB chunks through fp16 matmuls
against group-bilinear tiles built on-chip.  The hs/ts factors are
round-tripped through DRAM so per-group partition-replicated layouts
(hsdup / tsd) can be produced by plain DMAs instead of PE broadcasts.
"""

import numpy as np

import concourse.bacc as bacc
import concourse.bass as bass
import concourse.tile as tile
from concourse import mybir
from concourse.bass_utils import run_bass_kernel_spmd
from concourse.masks import make_identity

F32 = mybir.dt.float32
F16 = mybir.dt.float16

B, L, H = 2, 1024, 768
E, M = 32, 4
EMB, BLK, NL = 768, 64, 97
G = EMB // BLK  # 12
LN_EPS = 1e-12

N_CORES = 8
IB = E // (N_CORES // B)     # 8 i-rows per core
NPAIR = IB * E               # 256 pairs per core
PT = NPAIR // 128            # 2 pair-tiles
KT = EMB * BLK // 128        # 384 k-tiles
WCH = 16                     # k-tiles per W_ext DMA chunk (3 MB each)
BKT = 4                      # k-tiles per DVE bilinear batch
CT = EMB // 128              # 6 feature chunks
KC = H // 128                # 6 contraction chunks of H
LC = L // 128                # 8 chunks of L
NENT = IB + E + 1            # 41 cols: [my 8 entities | all 32 | cls]
NE2 = NENT + 1


def _build_module():
    nc = bacc.Bacc("TRN2", target_bir_lowering=False, debug=False)

    seq_d = nc.dram_tensor("seq", [L, H], F16, kind="ExternalInput")
    S_d = nc.dram_tensor("S", [L, NENT], F16, kind="ExternalInput")
    Wh_d = nc.dram_tensor("Wh", [3 * H, EMB], F16, kind="ExternalInput")
    Wt_d = nc.dram_tensor("Wt", [3 * H, EMB], F16, kind="ExternalInput")
    bh_d = nc.dram_tensor("bh", [128, CT], F32, kind="ExternalInput")
    bt_d = nc.dram_tensor("bt", [128, CT], F32, kind="ExternalInput")
    # W_ext repacked host-side to partition-major [p, kt, o] so each chunk
    # DMA is one fully-contiguous ~24KB line per partition.
    Wx_d = nc.dram_tensor("Wx", [128, KT * EMB], F16, kind="ExternalInput")
    bx_d = nc.dram_tensor("bx", [128, EMB], F32, kind="ExternalInput")
    lng_d = nc.dram_tensor("lng", [128, EMB], F32, kind="ExternalInput")
    lnb_d = nc.dram_tensor("lnb", [128, EMB], F32, kind="ExternalInput")
    Wc_d = nc.dram_tensor("Wc", [EMB, NL], F16, kind="ExternalInput")
    out_d = nc.dram_tensor("out", [NPAIR, NL], F32, kind="ExternalOutput")

    with tile.TileContext(nc) as tc:
        with (
            tc.tile_pool(name="persist", bufs=1) as persist,
            tc.tile_pool(name="seqp", bufs=1) as seqp,
            tc.tile_pool(name="whp", bufs=9) as whp,
            tc.tile_pool(name="wxp", bufs=3) as wxp,
            tc.tile_pool(name="blp", bufs=4) as blp,
            tc.tile_pool(name="hsdupp", bufs=2) as hsdupp,
            tc.tile_pool(name="tsdp", bufs=2) as tsdp,
            tc.tile_pool(name="hstp", bufs=4) as hstp,
            tc.tile_pool(name="tmpp", bufs=3) as tmpp,
            tc.tile_pool(name="dramp", bufs=1, space="DRAM") as dramp,
            tc.tile_pool(name="psf", bufs=1, space="PSUM") as psf,
            tc.tile_pool(name="psg", bufs=2, space="PSUM") as psg,
        ):
            ident = persist.tile([128, 128], F32, name="ident")
            make_identity(nc, ident[:])

            # ---- per-column constants broadcast to all partitions ----
            bx_b = persist.tile([128, EMB], F32, name="bx_b")
            lng_b = persist.tile([128, EMB], F32, name="lng_b")
            lnb_b = persist.tile([128, EMB], F32, name="lnb_b")
            for tile_, src in ((bx_b, bx_d), (lng_b, lng_d), (lnb_b, lnb_d)):
                nc.sync.dma_start(tile_[:], src.ap())

            eps_t = persist.tile([128, 1], F32, name="eps")
            nc.vector.memset(eps_t[:], LN_EPS)

            # per-partition bias chunks bh/bt: [128, CT]
            bh_t = persist.tile([128, CT], F32, name="bh_t")
            bt_t = persist.tile([128, CT], F32, name="bt_t")
            for tile_, src in ((bh_t, bh_d), (bt_t, bt_d)):
                nc.sync.dma_start(tile_[:], src.ap())

            # ---- phase E: entity pooling  ent = S^T @ seq ----
            seq_t = seqp.tile([128, LC, H], F16, name="seq_t")
            S_t = seqp.tile([128, LC, NENT], F16, name="S_t")
            seq_re = seq_d.ap().rearrange("(c p) h -> p c h", p=128)
            S_re = S_d.ap().rearrange("(c p) n -> p c n", p=128)
            for kc in range(LC):
                nc.sync.dma_start(S_t[:, kc, :], S_re[:, kc, :])
                nc.sync.dma_start(seq_t[:, kc, :], seq_re[:, kc, :])

            ps_e0 = psg.tile([NENT, 512], F32, name="gen")
            ps_e1 = psg.tile([NENT, 256], F32, name="gen")
            for kc in range(LC):
                nc.tensor.matmul(ps_e0[:], S_t[:, kc, :], seq_t[:, kc, 0:512],
                                 start=(kc == 0), stop=(kc == LC - 1))
                nc.tensor.matmul(ps_e1[:], S_t[:, kc, :], seq_t[:, kc, 512:768],
                                 start=(kc == 0), stop=(kc == LC - 1))
            ent_nat = persist.tile([NENT, H], F32, name="ent_nat")
            nc.vector.tensor_scalar_mul(ent_nat[:, 0:512], ps_e0[:], 1.0)
            nc.vector.tensor_scalar_mul(ent_nat[:, 512:768], ps_e1[:], 1.0)

            # transpose ent -> entT [h, NENT]  (feeds the projection matmuls)
            entT = persist.tile([128, KC, NENT], F16, name="entT")
            for kc in range(KC):
                ps_tr = psg.tile([128, NENT], F32, name="gen")
                nc.tensor.transpose(ps_tr[:], ent_nat[:, kc * 128:(kc + 1) * 128],
                                    ident[:NENT, :NENT])
                nc.vector.tensor_scalar_mul(entT[:, kc, :], ps_tr[:], 1.0)

            # ---- phase A: A/B/C projections ----
            # natural layout first: X_nat = ent @ W_block  [41, 768], then
            # PE-transpose into ABCD[ct][:, m, :] ([c,41], m: Ah,Bh,At,Bt).
            ABCD = []
            for ct in range(CT):
                abcd_alloc = persist.tile([128, 4, NE2], F32, name=f"abcd{ct}")
                nc.vector.memset(abcd_alloc[:], 0.0)
                ABCD.append(abcd_alloc)

            # feat accumulators allocated early: the ts-side phase-A chains
            # borrow these idle PSUM banks so chains can run concurrently.
            ps_feat = [[psf.tile([128, 512], F32, name=f"pf{pt}a"),
                        psf.tile([128, 256], F32, name=f"pf{pt}b")]
                       for pt in range(PT)]

            def emit_ab_chain(m, w_d, blk, ps_pair=None):
                if ps_pair is None:
                    ps_n0 = psg.tile([NENT, 512], F32, name="gen")
                    ps_n1 = psg.tile([NENT, 256], F32, name="gen")
                else:
                    ps_n0 = ps_pair[0][:NENT, :]
                    ps_n1 = ps_pair[1][:NENT, :]
                for kc in range(KC):
                    w_t = whp.tile([128, EMB], F16, name="w_t")
                    nc.sync.dma_start(
                        w_t[:], w_d.ap()[blk * H + kc * 128: blk * H + (kc + 1) * 128, :])
                    nc.tensor.matmul(ps_n0[:], entT[:, kc, :], w_t[:, 0:512],
                                     start=(kc == 0), stop=(kc == KC - 1))
                    nc.tensor.matmul(ps_n1[:], entT[:, kc, :], w_t[:, 512:768],
                                     start=(kc == 0), stop=(kc == KC - 1))
                x_nat = tmpp.tile([NENT, EMB], F32, name="x_nat")
                nc.vector.tensor_scalar_mul(x_nat[:, 0:512], ps_n0[:], 1.0)
                nc.vector.tensor_scalar_mul(x_nat[:, 512:768], ps_n1[:], 1.0)
                for ct in range(CT):
                    ps_tr = psg.tile([128, NENT], F32, name="gen")
                    nc.tensor.transpose(ps_tr[:], x_nat[:, ct * 128:(ct + 1) * 128],
                                        ident[:NENT, :NENT])
                    nc.vector.tensor_scalar_mul(ABCD[ct][:, m, 0:NENT], ps_tr[:], 1.0)

            def emit_c_chain(m_sel, w_d, bias_t):
                ps_c0 = psg.tile([NENT, 512], F32, name="gen")
                ps_c1 = psg.tile([NENT, 256], F32, name="gen")
                for kc in range(KC):
                    w_t = whp.tile([128, EMB], F16, name="w_t")
                    nc.sync.dma_start(
                        w_t[:], w_d.ap()[2 * H + kc * 128: 2 * H + (kc + 1) * 128, :])
                    nc.tensor.matmul(ps_c0[:1, :], entT[:, kc, IB + E:IB + E + 1],
                                     w_t[:, 0:512],
                                     start=(kc == 0), stop=(kc == KC - 1))
                    nc.tensor.matmul(ps_c1[:1, :], entT[:, kc, IB + E:IB + E + 1],
                                     w_t[:, 512:768],
                                     start=(kc == 0), stop=(kc == KC - 1))
                c_nat = tmpp.tile([1, EMB], F32, name="c_nat")
                nc.vector.tensor_scalar_mul(c_nat[:, 0:512], ps_c0[:1, :], 1.0)
                nc.vector.tensor_scalar_mul(c_nat[:, 512:768], ps_c1[:1, :], 1.0)
                for ct in range(CT):
                    ps_tr = psg.tile([128, NENT], F32, name="gen")
                    nc.tensor.transpose(ps_tr[:, 0:1],
                                        c_nat[:, ct * 128:(ct + 1) * 128],
                                        ident[:1, :1])
                    nc.vector.tensor_tensor(ABCD[ct][:, m_sel, NENT:NENT + 1],
                                            ps_tr[:, 0:1],
                                            bias_t[:, ct:ct + 1],
                                            op=mybir.AluOpType.add)

            def colview(tile_, m, col0, ap_pat):
                return bass.AP(tensor=tile_.tensor,
                               offset=tile_.offset + m * NE2 + col0,
                               ap=[tile_.ap[0]] + ap_pat)

            # DRAM round-trip buffers for the pair factors.  k-tiles tile a
            # group's 64x64 (i,j) grid as [8 di x 16 pj] so hs replicates
            # 16x and ts 8x (24x total vs 66x for [2 x 64]).
            # ts_dram: natural feature order [768 rows, 256].
            # hs_dram: digit-swapped order: row (g*64 + di*8 + ib2) holds
            #   hs feature (g*64 + ib2*8 + di) -> the 8 rows a (g, di)
            #   read needs are one contiguous 4KB block.
            ts_dram = dramp.tile([EMB, 256], F16, name="ts_dram")
            hs_dram = dramp.tile([EMB, 256], F16, name="hs_dram")

            def emit_tanh(ct, ma, mb, cola, colb, msel, dst_dram, dup_order):
                # x[p, il, j] = A[p, cola+?] + B[p, ?]  (see colview patterns)
                tmp = tmpp.tile([128, 8, 32], F32, name="tmp")
                nc.vector.tensor_tensor(
                    tmp[:], colview(ABCD[ct], ma, cola[0], cola[1]),
                    colview(ABCD[ct], mb, colb[0], colb[1]),
                    op=mybir.AluOpType.add)
                xt = hstp.tile([128, 256], F16, name="xt")
                nc.scalar.activation(
                    xt[:].rearrange("p (a b) -> p a b", a=8),
                    tmp[:], mybir.ActivationFunctionType.Tanh,
                    bias=ABCD[ct][:, msel, NENT:NENT + 1], scale=1.0)
                if dup_order:
                    # partition p = ph*64 + ib2*8 + di (feature ct*128+p)
                    # -> row g*64 + di*8 + ib2 with g = 2ct + ph; one DMA
                    # per ph half keeps the dst AP at 3 dims.
                    for ph in range(2):
                        dst = bass.AP(
                            tensor=dst_dram.tensor,
                            offset=dst_dram.offset + (ct * 128 + ph * 64) * 256,
                            ap=[[256, 8], [8 * 256, 8], [1, 256]])
                        nc.scalar.dma_start(dst, xt[ph * 64:(ph + 1) * 64, :])
                else:
                    nc.scalar.dma_start(dst_dram[ct * 128:(ct + 1) * 128, :], xt[:])

            # ---- projections (all six chains, PE-dense) ----
            emit_c_chain(3, Wt_d, bt_t)
            emit_ab_chain(2, Wt_d, 0, ps_feat[0])
            emit_ab_chain(3, Wt_d, 1, ps_feat[1])
            emit_c_chain(0, Wh_d, bh_t)
            emit_ab_chain(0, Wh_d, 0, ps_feat[0])
            emit_ab_chain(1, Wh_d, 1, ps_feat[1])

            # classifier weights + fp16 identity staged early so phase L
            # has no DMA dependency at the tail.
            wc_t = persist.tile([128, CT, NL], F16, name="wc_t")
            nc.sync.dma_start(wc_t[:], Wc_d.ap().rearrange("(c p) n -> p c n", p=128))
            ident16 = persist.tile([128, 128], F16, name="ident16")
            nc.scalar.copy(ident16[:], ident[:])

            def emit_tanh_ct(ct):
                # ts[pair=(il,j)] = tanh(At[j] + Bt[il] + Ct + bt)
                emit_tanh(ct, 2, 3, (IB, [[0, 8], [1, 32]]), (0, [[1, 8], [0, 32]]),
                          3, ts_dram, dup_order=False)
                # hs[pair=(il,j)] = tanh(Ah[il] + Bh[j] + Ch + bh)
                emit_tanh(ct, 0, 1, (0, [[1, 8], [0, 32]]), (IB, [[0, 8], [1, 32]]),
                          0, hs_dram, dup_order=True)

            emit_tanh_ct(0)

            # ---- phase M: main contraction over W_ext ----
            # k-tile (g, ib2, jb): partition p = di*16 + pj covers k-row
            # g*4096 + (ib2*8+di)*64 + jb*16 + pj.  Factor tiles per group:
            # hsdup[p, ib2, pair] = hs[g*64 + ib2*8 + p//16, pair]
            # tsdup[p, jb, pair]  = ts[g*64 + jb*16 + p%16, pair]
            wx_ch = None
            for g in range(G):
                # stay one ct ahead of the groups that consume it
                ct_next = g // 2 + 1
                if g % 2 == 0 and ct_next < CT:
                    emit_tanh_ct(ct_next)

                hsdup = hsdupp.tile([128, 8, 256], F16, name="hsdup")
                for di in range(8):
                    src = bass.AP(
                        tensor=hs_dram.tensor,
                        offset=hs_dram.offset + (g * 64 + di * 8) * 256,
                        ap=[[0, 16], [1, 8 * 256]])
                    nc.scalar.dma_start(
                        hsdup[di * 16:(di + 1) * 16, :, :].rearrange(
                            "p l c -> p (l c)"), src)
                tsdup = tsdp.tile([128, 4, 256], F16, name="tsdup")
                for di in range(8):
                    src = bass.AP(
                        tensor=ts_dram.tensor,
                        offset=ts_dram.offset + g * 64 * 256,
                        ap=[[256, 16], [16 * 256, 4], [1, 256]])
                    nc.sync.dma_start(tsdup[di * 16:(di + 1) * 16, :, :], src)

                for ib2 in range(8):
                    kt0 = g * 32 + ib2 * 4
                    if kt0 % WCH == 0:
                        wx_ch = wxp.tile([128, WCH * EMB], F16, name="wx_ch")
                        c0 = kt0 * EMB
                        nc.sync.dma_start(wx_ch[:], Wx_d.ap()[:, c0:c0 + WCH * EMB])

                    bl4 = blp.tile([128, 4, 256], F16, name="bl4")
                    hs_b = bass.AP(tensor=hsdup.tensor,
                                   offset=hsdup.offset + ib2 * 256,
                                   ap=[hsdup.ap[0], [0, 4], [1, 256]])
                    nc.vector.tensor_tensor(bl4[:], hs_b, tsdup[:],
                                            op=mybir.AluOpType.mult)
                    for jb in range(4):
                        kt = kt0 + jb
                        kl = kt % WCH
                        for pt in range(PT):
                            lhsT = bl4[:, jb, pt * 128:(pt + 1) * 128]
                            nc.tensor.matmul(
                                ps_feat[pt][0][:], lhsT,
                                wx_ch[:, kl * EMB:kl * EMB + 512],
                                start=(kt == 0), stop=(kt == KT - 1))
                            nc.tensor.matmul(
                                ps_feat[pt][1][:], lhsT,
                                wx_ch[:, kl * EMB + 512:(kl + 1) * EMB],
                                start=(kt == 0), stop=(kt == KT - 1))

            # ---- phase L: bias, relu, layernorm, classifier ----
            for pt in range(PT):
                feat = persist.tile([128, EMB], F32, name=f"feat{pt}")
                nc.vector.tensor_tensor(feat[:, 0:512], ps_feat[pt][0][:],
                                        bx_b[:, 0:512], op=mybir.AluOpType.add)
                nc.vector.tensor_tensor(feat[:, 512:768], ps_feat[pt][1][:],
                                        bx_b[:, 512:768], op=mybir.AluOpType.add)
                nc.scalar.activation(feat[:], feat[:],
                                     mybir.ActivationFunctionType.Relu,
                                     bias=0.0, scale=1.0)

                stats = tmpp.tile([128, 3, 6], F32, name="stats")
                f_re = feat.rearrange("p (c f) -> p c f", c=3)
                for c in range(3):
                    nc.vector.bn_stats(stats[:, c, :], f_re[:, c, :])
                mv = tmpp.tile([128, 2], F32, name="mv")
                nc.vector.bn_aggr(mv[:], stats[:])
                sd = tmpp.tile([128, 1], F32, name="sd")
                nc.scalar.activation(sd[:], mv[:, 1:2],
                                     mybir.ActivationFunctionType.Sqrt,
                                     bias=eps_t[:], scale=1.0)
                rstd = tmpp.tile([128, 1], F32, name="rstd")
                nc.vector.reciprocal(rstd[:], sd[:])

                lnf = persist.tile([128, EMB], F32, name=f"lnf{pt}")
                nc.vector.tensor_scalar(lnf[:], feat[:], mv[:, 0:1], rstd[:],
                                        op0=mybir.AluOpType.subtract,
                                        op1=mybir.AluOpType.mult)
                nc.vector.tensor_tensor(lnf[:], lnf[:], lng_b[:],
                                        op=mybir.AluOpType.mult)
                ln = persist.tile([128, EMB], F16, name=f"ln{pt}")
                nc.vector.tensor_tensor(ln[:], lnf[:], lnb_b[:],
                                        op=mybir.AluOpType.add)

                lnT = persist.tile([128, CT, 128], F16, name=f"lnT{pt}")
                for ct in range(CT):
                    ps_tr2 = psg.tile([128, 128], F16, name="gen16")
                    nc.tensor.transpose(ps_tr2[:], ln[:, ct * 128:(ct + 1) * 128],
                                        ident16[:])
                    nc.vector.tensor_scalar_mul(lnT[:, ct, :], ps_tr2[:], 1.0)

                ps_lg = psg.tile([128, NL], F32, name="gen")
                for ct in range(CT):
                    nc.tensor.matmul(ps_lg[:], lnT[:, ct, :], wc_t[:, ct, :],
                                     start=(ct == 0), stop=(ct == CT - 1))
                out_sb = tmpp.tile([128, NL], F32, name="out_sb")
                nc.vector.tensor_scalar_mul(out_sb[:], ps_lg[:], 1.0)
                nc.scalar.dma_start(out_d.ap()[pt * 128:(pt + 1) * 128, :], out_sb[:])

    nc.compile()
    return nc


_NC_CACHE = []


def _get_module():
    if not _NC_CACHE:
        _NC_CACHE.append(_build_module())
    return _NC_CACHE[0]


def _build_inputs(seq, starts, ends, mention_mask, W_head, b_head, W_tail, b_tail,
                  W_ext, b_ext, ln_g, ln_b, W_cls):
    seq = np.asarray(seq, np.float32)
    starts = np.asarray(starts, np.int64)
    ends = np.asarray(ends, np.int64)
    mask = np.asarray(mention_mask, np.float32)

    # per-document entity selection matrix: ent = Sb^T @ seq[b]
    S_b = np.zeros((B, L, E), np.float32)
    denom = np.maximum(mask.sum(axis=2), 1.0)          # [B, E]
    w = mask * 0.5 / denom[:, :, None]                 # [B, E, M]
    for b in range(B):
        for e in range(E):
            np.add.at(S_b[b, :, e], starts[b, e] + 1, w[b, e])
            np.add.at(S_b[b, :, e], ends[b, e], w[b, e])

    cls_col = np.zeros((L, 1), np.float32)
    cls_col[0, 0] = 1.0

    shared = {
        "Wh": np.ascontiguousarray(np.asarray(W_head, np.float32).astype(np.float16)),
        "Wt": np.ascontiguousarray(np.asarray(W_tail, np.float32).astype(np.float16)),
        "bh": np.ascontiguousarray(np.asarray(b_head, np.float32).reshape(CT, 128).T),
        "bt": np.ascontiguousarray(np.asarray(b_tail, np.float32).reshape(CT, 128).T),
        # partition p = di*16+pj, kt = (g, ib2, jb); row k = g*4096 +
        # (ib2*8+di)*64 + jb*16 + pj
        "Wx": np.ascontiguousarray(
            np.asarray(W_ext).astype(np.float16)
            .reshape(G, 8, 8, 4, 16, EMB).transpose(2, 4, 0, 1, 3, 5)
            .reshape(128, KT * EMB)),
        "bx": np.ascontiguousarray(np.broadcast_to(np.asarray(b_ext, np.float32), (128, EMB))),
        "lng": np.ascontiguousarray(np.broadcast_to(np.asarray(ln_g, np.float32), (128, EMB))),
        "lnb": np.ascontiguousarray(np.broadcast_to(np.asarray(ln_b, np.float32), (128, EMB))),
        "Wc": np.ascontiguousarray(np.asarray(W_cls, np.float32).astype(np.float16)),
    }
    in_maps = []
    for core in range(N_CORES):
        b, ib = core // 4, core % 4
        S_core = np.concatenate(
            [S_b[b][:, ib * IB:(ib + 1) * IB], S_b[b], cls_col], axis=1)
        in_maps.append({
            "seq": np.ascontiguousarray(seq[b].astype(np.float16)),
            "S": np.ascontiguousarray(S_core.astype(np.float16)),
            **shared,
        })
    return in_maps


def kernel(**inputs) -> np.ndarray:
    nc = _get_module()
    in_maps = _build_inputs(**inputs)
    res = run_bass_kernel_spmd(nc, in_maps, core_ids=list(range(N_CORES)))
    outs = np.stack([res.results[c]["out"] for c in range(N_CORES)])  # [8,256,97]
    return outs.reshape(B, 4, IB, E, NL).reshape(B, E, E, NL)


# revision 34
# speedup vs baseline: 1.2255x; 1.0450x over previous
"""DocRE model kernel for 8 Trainium2 NeuronCores.

Data-parallel over the pair grid: core = b*4 + ib owns document b and
i-rows [8*ib, 8*ib+8) of the 32x32 entity-pair grid (256 pairs/core).
All weights are replicated; W_ext (49152x768, repacked partition-major
on the host) is streamed from HBM in 3MB chunks through fp16 matmuls
against group-bilinear tiles built on-chip.  The hs/ts factors are
round-tripped through DRAM so per-group partition-replicated layouts
(hsdup / tsd) can be produced by plain DMAs instead of PE broadcasts.
"""

import numpy as np

import concourse.bacc as bacc
import concourse.bass as bass
import concourse.tile as tile
from concourse import mybir
from concourse.bass_utils import run_bass_kernel_spmd
from concourse.masks import make_identity

F32 = mybir.dt.float32
F16 = mybir.dt.float16

B, L, H = 2, 1024, 768
E, M = 32, 4
EMB, BLK, NL = 768, 64, 97
G = EMB // BLK  # 12
LN_EPS = 1e-12

N_CORES = 8
IB = E // (N_CORES // B)     # 8 i-rows per core
NPAIR = IB * E               # 256 pairs per core
PT = NPAIR // 128            # 2 pair-tiles
KT = EMB * BLK // 128        # 384 k-tiles
WCH = 16                     # k-tiles per W_ext DMA chunk (3 MB each)
BKT = 4                      # k-tiles per DVE bilinear batch
CT = EMB // 128              # 6 feature chunks
KC = H // 128                # 6 contraction chunks of H
LC = L // 128                # 8 chunks of L
NENT = IB + E + 1            # 41 cols: [my 8 entities | all 32 | cls]
NE2 = NENT + 1


def _build_module():
    nc = bacc.Bacc("TRN2", target_bir_lowering=False, debug=False)

    seq_d = nc.dram_tensor("seq", [L, H], F16, kind="ExternalInput")
    S_d = nc.dram_tensor("S", [L, NENT], F16, kind="ExternalInput")
    Wh_d = nc.dram_tensor("Wh", [3 * H, EMB], F16, kind="ExternalInput")
    Wt_d = nc.dram_tensor("Wt", [3 * H, EMB], F16, kind="ExternalInput")
    bh_d = nc.dram_tensor("bh", [128, CT], F32, kind="ExternalInput")
    bt_d = nc.dram_tensor("bt", [128, CT], F32, kind="ExternalInput")
    # W_ext repacked host-side to partition-major [p, kt, o] so each chunk
    # DMA is one fully-contiguous ~24KB line per partition.
    Wx_d = nc.dram_tensor("Wx", [128, KT * EMB], F16, kind="ExternalInput")
    bx_d = nc.dram_tensor("bx", [128, EMB], F32, kind="ExternalInput")
    lng_d = nc.dram_tensor("lng", [128, EMB], F32, kind="ExternalInput")
    lnb_d = nc.dram_tensor("lnb", [128, EMB], F32, kind="ExternalInput")
    Wc_d = nc.dram_tensor("Wc", [EMB, NL], F16, kind="ExternalInput")
    out_d = nc.dram_tensor("out", [NPAIR, NL], F32, kind="ExternalOutput")

    with tile.TileContext(nc) as tc:
        with (
            tc.tile_pool(name="persist", bufs=1) as persist,
            tc.tile_pool(name="seqp", bufs=1) as seqp,
            tc.tile_pool(name="whp", bufs=3) as whp,
            tc.tile_pool(name="wxp", bufs=3) as wxp,
            tc.tile_pool(name="blp", bufs=6) as blp,
            tc.tile_pool(name="hsdupp", bufs=3) as hsdupp,
            tc.tile_pool(name="tsdp", bufs=3) as tsdp,
            tc.tile_pool(name="hstp", bufs=4) as hstp,
            tc.tile_pool(name="tmpp", bufs=3) as tmpp,
            tc.tile_pool(name="dramp", bufs=1, space="DRAM") as dramp,
            tc.tile_pool(name="psf", bufs=1, space="PSUM") as psf,
            tc.tile_pool(name="psg", bufs=2, space="PSUM") as psg,
        ):
            ident = persist.tile([128, 128], F32, name="ident")
            make_identity(nc, ident[:])

            # ---- per-column constants broadcast to all partitions ----
            bx_b = persist.tile([128, EMB], F32, name="bx_b")
            lng_b = persist.tile([128, EMB], F32, name="lng_b")
            lnb_b = persist.tile([128, EMB], F32, name="lnb_b")
            for tile_, src in ((bx_b, bx_d), (lng_b, lng_d), (lnb_b, lnb_d)):
                nc.sync.dma_start(tile_[:], src.ap())

            eps_t = persist.tile([128, 1], F32, name="eps")
            nc.vector.memset(eps_t[:], LN_EPS)

            # per-partition bias chunks bh/bt: [128, CT]
            bh_t = persist.tile([128, CT], F32, name="bh_t")
            bt_t = persist.tile([128, CT], F32, name="bt_t")
            for tile_, src in ((bh_t, bh_d), (bt_t, bt_d)):
                nc.sync.dma_start(tile_[:], src.ap())

            # ---- phase E: entity pooling  ent = S^T @ seq ----
            seq_t = seqp.tile([128, LC, H], F16, name="seq_t")
            S_t = seqp.tile([128, LC, NENT], F16, name="S_t")
            seq_re = seq_d.ap().rearrange("(c p) h -> p c h", p=128)
            S_re = S_d.ap().rearrange("(c p) n -> p c n", p=128)
            nc.sync.dma_start(S_t[:], S_re)
            nc.sync.dma_start(seq_t[:], seq_re)

            ps_e0 = psg.tile([NENT, 512], F32, name="gen")
            ps_e1 = psg.tile([NENT, 256], F32, name="gen")
            for kc in range(LC):
                nc.tensor.matmul(ps_e0[:], S_t[:, kc, :], seq_t[:, kc, 0:512],
                                 start=(kc == 0), stop=(kc == LC - 1))
                nc.tensor.matmul(ps_e1[:], S_t[:, kc, :], seq_t[:, kc, 512:768],
                                 start=(kc == 0), stop=(kc == LC - 1))
            ent_nat = persist.tile([NENT, H], F32, name="ent_nat")
            nc.vector.tensor_scalar_mul(ent_nat[:, 0:512], ps_e0[:], 1.0)
            nc.vector.tensor_scalar_mul(ent_nat[:, 512:768], ps_e1[:], 1.0)

            # transpose ent -> entT [h, NENT]  (feeds the projection matmuls)
            entT = persist.tile([128, KC, NENT], F16, name="entT")
            for kc in range(KC):
                ps_tr = psg.tile([128, NENT], F32, name="gen")
                nc.tensor.transpose(ps_tr[:], ent_nat[:, kc * 128:(kc + 1) * 128],
                                    ident[:NENT, :NENT])
                nc.vector.tensor_scalar_mul(entT[:, kc, :], ps_tr[:], 1.0)

            # ---- phase A: A/B/C projections ----
            # natural layout first: X_nat = ent @ W_block  [41, 768], then
            # PE-transpose into ABCD[ct][:, m, :] ([c,41], m: Ah,Bh,At,Bt).
            ABCD = []
            for ct in range(CT):
                abcd_alloc = persist.tile([128, 4, NE2], F32, name=f"abcd{ct}")
                nc.vector.memset(abcd_alloc[:], 0.0)
                ABCD.append(abcd_alloc)

            # feat accumulators allocated early: the ts-side phase-A chains
            # borrow these idle PSUM banks so chains can run concurrently.
            ps_feat = [[psf.tile([128, 512], F32, name=f"pf{pt}a"),
                        psf.tile([128, 256], F32, name=f"pf{pt}b")]
                       for pt in range(PT)]

            def emit_ab_chain(m, w_d, blk, ps_pair=None):
                if ps_pair is None:
                    ps_n0 = psg.tile([NENT, 512], F32, name="gen")
                    ps_n1 = psg.tile([NENT, 256], F32, name="gen")
                else:
                    ps_n0 = ps_pair[0][:NENT, :]
                    ps_n1 = ps_pair[1][:NENT, :]
                w_t = whp.tile([128, KC, EMB], F16, name="w_t")
                nc.sync.dma_start(
                    w_t[:], w_d.ap()[blk * H:(blk + 1) * H, :].rearrange(
                        "(c p) h -> p c h", p=128))
                for kc in range(KC):
                    nc.tensor.matmul(ps_n0[:], entT[:, kc, :], w_t[:, kc, 0:512],
                                     start=(kc == 0), stop=(kc == KC - 1))
                    nc.tensor.matmul(ps_n1[:], entT[:, kc, :], w_t[:, kc, 512:768],
                                     start=(kc == 0), stop=(kc == KC - 1))
                x_nat = tmpp.tile([NENT, EMB], F32, name="x_nat")
                nc.vector.tensor_scalar_mul(x_nat[:, 0:512], ps_n0[:], 1.0)
                nc.vector.tensor_scalar_mul(x_nat[:, 512:768], ps_n1[:], 1.0)
                for ct in range(CT):
                    ps_tr = psg.tile([128, NENT], F32, name="gen")
                    nc.tensor.transpose(ps_tr[:], x_nat[:, ct * 128:(ct + 1) * 128],
                                        ident[:NENT, :NENT])
                    nc.vector.tensor_scalar_mul(ABCD[ct][:, m, 0:NENT], ps_tr[:], 1.0)

            def emit_c_chain(m_sel, w_d, bias_t):
                ps_c0 = psg.tile([NENT, 512], F32, name="gen")
                ps_c1 = psg.tile([NENT, 256], F32, name="gen")
                w_t = whp.tile([128, KC, EMB], F16, name="w_t")
                nc.sync.dma_start(
                    w_t[:], w_d.ap()[2 * H:3 * H, :].rearrange(
                        "(c p) h -> p c h", p=128))
                for kc in range(KC):
                    nc.tensor.matmul(ps_c0[:1, :], entT[:, kc, IB + E:IB + E + 1],
                                     w_t[:, kc, 0:512],
                                     start=(kc == 0), stop=(kc == KC - 1))
                    nc.tensor.matmul(ps_c1[:1, :], entT[:, kc, IB + E:IB + E + 1],
                                     w_t[:, kc, 512:768],
                                     start=(kc == 0), stop=(kc == KC - 1))
                c_nat = tmpp.tile([1, EMB], F32, name="c_nat")
                nc.vector.tensor_scalar_mul(c_nat[:, 0:512], ps_c0[:1, :], 1.0)
                nc.vector.tensor_scalar_mul(c_nat[:, 512:768], ps_c1[:1, :], 1.0)
                for ct in range(CT):
                    ps_tr = psg.tile([128, NENT], F32, name="gen")
                    nc.tensor.transpose(ps_tr[:, 0:1],
                                        c_nat[:, ct * 128:(ct + 1) * 128],
                                        ident[:1, :1])
                    nc.vector.tensor_tensor(ABCD[ct][:, m_sel, NENT:NENT + 1],
                                            ps_tr[:, 0:1],
                                            bias_t[:, ct:ct + 1],
                                            op=mybir.AluOpType.add)

            def colview(tile_, m, col0, ap_pat):
                return bass.AP(tensor=tile_.tensor,
                               offset=tile_.offset + m * NE2 + col0,
                               ap=[tile_.ap[0]] + ap_pat)

            # DRAM round-trip buffers for the pair factors.  k-tiles tile a
            # group's 64x64 (i,j) grid as [8 di x 16 pj] so hs replicates
            # 16x and ts 8x (24x total vs 66x for [2 x 64]).
            # ts_dram: natural feature order [768 rows, 256].
            # hs_dram: digit-swapped order: row (g*64 + di*8 + ib2) holds
            #   hs feature (g*64 + ib2*8 + di) -> the 8 rows a (g, di)
            #   read needs are one contiguous 4KB block.
            ts_dram = dramp.tile([EMB, 256], F16, name="ts_dram")
            hs_dram = dramp.tile([EMB, 256], F16, name="hs_dram")

            def emit_tanh(ct, ma, mb, cola, colb, msel, dst_dram, dup_order):
                # x[p, il, j] = A[p, cola+?] + B[p, ?]  (see colview patterns)
                tmp = tmpp.tile([128, 8, 32], F32, name="tmp")
                nc.vector.tensor_tensor(
                    tmp[:], colview(ABCD[ct], ma, cola[0], cola[1]),
                    colview(ABCD[ct], mb, colb[0], colb[1]),
                    op=mybir.AluOpType.add)
                xt = hstp.tile([128, 256], F16, name="xt")
                nc.scalar.activation(
                    xt[:].rearrange("p (a b) -> p a b", a=8),
                    tmp[:], mybir.ActivationFunctionType.Tanh,
                    bias=ABCD[ct][:, msel, NENT:NENT + 1], scale=1.0)
                if dup_order:
                    # partition p = ph*64 + ib2*8 + di (feature ct*128+p)
                    # -> row g*64 + di*8 + ib2 with g = 2ct + ph; one DMA
                    # per ph half keeps the dst AP at 3 dims.
                    for ph in range(2):
                        dst = bass.AP(
                            tensor=dst_dram.tensor,
                            offset=dst_dram.offset + (ct * 128 + ph * 64) * 256,
                            ap=[[256, 8], [8 * 256, 8], [1, 256]])
                        nc.scalar.dma_start(dst, xt[ph * 64:(ph + 1) * 64, :])
                else:
                    nc.scalar.dma_start(dst_dram[ct * 128:(ct + 1) * 128, :], xt[:])

            # ---- projections (six chains, PE-dense) ----

            # classifier weights + fp16 identity staged early so phase L
            # has no DMA dependency at the tail.
            wc_t = persist.tile([128, CT, NL], F16, name="wc_t")
            nc.sync.dma_start(wc_t[:], Wc_d.ap().rearrange("(c p) n -> p c n", p=128))
            ident16 = persist.tile([128, 128], F16, name="ident16")
            nc.scalar.copy(ident16[:], ident[:])

            def emit_tanh_ts(ct):
                # ts[pair=(il,j)] = tanh(At[j] + Bt[il] + Ct + bt)
                emit_tanh(ct, 2, 3, (IB, [[0, 8], [1, 32]]), (0, [[1, 8], [0, 32]]),
                          3, ts_dram, dup_order=False)

            def emit_tanh_hs(ct):
                # hs[pair=(il,j)] = tanh(Ah[il] + Bh[j] + Ch + bh)
                emit_tanh(ct, 0, 1, (0, [[1, 8], [0, 32]]), (IB, [[0, 8], [1, 32]]),
                          0, hs_dram, dup_order=True)

            def emit_tanh_ct(ct):
                emit_tanh_ts(ct)
                emit_tanh_hs(ct)

            # ts-side chains first so ts tanh/stores overlap the hs chains.
            emit_c_chain(3, Wt_d, bt_t)
            emit_ab_chain(2, Wt_d, 0, ps_feat[0])
            emit_ab_chain(3, Wt_d, 1, ps_feat[1])
            emit_tanh_ts(0)
            emit_c_chain(0, Wh_d, bh_t)
            emit_ab_chain(0, Wh_d, 0, ps_feat[0])
            emit_ab_chain(1, Wh_d, 1, ps_feat[1])
            emit_tanh_hs(0)

            # ---- phase M: main contraction over W_ext ----
            # k-tile (g, ib2, jb): partition p = di*16 + pj covers k-row
            # g*4096 + (ib2*8+di)*64 + jb*16 + pj.  Factor tiles per group:
            # hsdup[p, ib2, pair] = hs[g*64 + ib2*8 + p//16, pair]
            # tsdup[p, jb, pair]  = ts[g*64 + jb*16 + p%16, pair]
            wx_ch = None
            for g in range(G):
                # stay one ct ahead of the groups that consume it
                ct_next = g // 2 + 1
                if g % 2 == 0 and ct_next < CT:
                    emit_tanh_ct(ct_next)

                hsdup = hsdupp.tile([128, 8, 256], F16, name="hsdup")
                for di in range(8):
                    src = bass.AP(
                        tensor=hs_dram.tensor,
                        offset=hs_dram.offset + (g * 64 + di * 8) * 256,
                        ap=[[0, 16], [1, 8 * 256]])
                    nc.scalar.dma_start(
                        hsdup[di * 16:(di + 1) * 16, :, :].rearrange(
                            "p l c -> p (l c)"), src)
                tsdup = tsdp.tile([128, 4, 256], F16, name="tsdup")
                for di in range(8):
                    src = bass.AP(
                        tensor=ts_dram.tensor,
                        offset=ts_dram.offset + g * 64 * 256,
                        ap=[[256, 16], [16 * 256, 4], [1, 256]])
                    nc.sync.dma_start(tsdup[di * 16:(di + 1) * 16, :, :], src)

                for ib2 in range(8):
                    kt0 = g * 32 + ib2 * 4
                    if kt0 % WCH == 0:
                        wx_ch = wxp.tile([128, WCH * EMB], F16, name="wx_ch")
                        c0 = kt0 * EMB
                        nc.sync.dma_start(wx_ch[:], Wx_d.ap()[:, c0:c0 + WCH * EMB])

                    bl4 = blp.tile([128, 4, 256], F16, name="bl4")
                    hs_b = bass.AP(tensor=hsdup.tensor,
                                   offset=hsdup.offset + ib2 * 256,
                                   ap=[hsdup.ap[0], [0, 4], [1, 256]])
                    nc.vector.tensor_tensor(bl4[:], hs_b, tsdup[:],
                                            op=mybir.AluOpType.mult)
                    for jb in range(4):
                        kt = kt0 + jb
                        kl = kt % WCH
                        for pt in range(PT):
                            lhsT = bl4[:, jb, pt * 128:(pt + 1) * 128]
                            nc.tensor.matmul(
                                ps_feat[pt][0][:], lhsT,
                                wx_ch[:, kl * EMB:kl * EMB + 512],
                                start=(kt == 0), stop=(kt == KT - 1))
                            nc.tensor.matmul(
                                ps_feat[pt][1][:], lhsT,
                                wx_ch[:, kl * EMB + 512:(kl + 1) * EMB],
                                start=(kt == 0), stop=(kt == KT - 1))

            # ---- phase L: bias, relu, layernorm, classifier ----
            for pt in range(PT):
                feat = persist.tile([128, EMB], F32, name=f"feat{pt}")
                nc.vector.tensor_tensor(feat[:, 0:512], ps_feat[pt][0][:],
                                        bx_b[:, 0:512], op=mybir.AluOpType.add)
                nc.vector.tensor_tensor(feat[:, 512:768], ps_feat[pt][1][:],
                                        bx_b[:, 512:768], op=mybir.AluOpType.add)
                nc.scalar.activation(feat[:], feat[:],
                                     mybir.ActivationFunctionType.Relu,
                                     bias=0.0, scale=1.0)

                stats = tmpp.tile([128, 3, 6], F32, name="stats")
                f_re = feat.rearrange("p (c f) -> p c f", c=3)
                for c in range(3):
                    nc.vector.bn_stats(stats[:, c, :], f_re[:, c, :])
                mv = tmpp.tile([128, 2], F32, name="mv")
                nc.vector.bn_aggr(mv[:], stats[:])
                sd = tmpp.tile([128, 1], F32, name="sd")
                nc.scalar.activation(sd[:], mv[:, 1:2],
                                     mybir.ActivationFunctionType.Sqrt,
                                     bias=eps_t[:], scale=1.0)
                rstd = tmpp.tile([128, 1], F32, name="rstd")
                nc.vector.reciprocal(rstd[:], sd[:])

                lnf = persist.tile([128, EMB], F32, name=f"lnf{pt}")
                nc.vector.tensor_scalar(lnf[:], feat[:], mv[:, 0:1], rstd[:],
                                        op0=mybir.AluOpType.subtract,
                                        op1=mybir.AluOpType.mult)
                nc.vector.tensor_tensor(lnf[:], lnf[:], lng_b[:],
                                        op=mybir.AluOpType.mult)
                ln = persist.tile([128, EMB], F16, name=f"ln{pt}")
                nc.vector.tensor_tensor(ln[:], lnf[:], lnb_b[:],
                                        op=mybir.AluOpType.add)

                lnT = persist.tile([128, CT, 128], F16, name=f"lnT{pt}")
                for ct in range(CT):
                    ps_tr2 = psg.tile([128, 128], F16, name="gen16")
                    nc.tensor.transpose(ps_tr2[:], ln[:, ct * 128:(ct + 1) * 128],
                                        ident16[:])
                    nc.vector.tensor_scalar_mul(lnT[:, ct, :], ps_tr2[:], 1.0)

                ps_lg = psg.tile([128, NL], F32, name="gen")
                for ct in range(CT):
                    nc.tensor.matmul(ps_lg[:], lnT[:, ct, :], wc_t[:, ct, :],
                                     start=(ct == 0), stop=(ct == CT - 1))
                out_sb = tmpp.tile([128, NL], F32, name="out_sb")
                nc.vector.tensor_scalar_mul(out_sb[:], ps_lg[:], 1.0)
                nc.scalar.dma_start(out_d.ap()[pt * 128:(pt + 1) * 128, :], out_sb[:])

    nc.compile()
    return nc


_NC_CACHE = []


def _get_module():
    if not _NC_CACHE:
        _NC_CACHE.append(_build_module())
    return _NC_CACHE[0]


def _build_inputs(seq, starts, ends, mention_mask, W_head, b_head, W_tail, b_tail,
                  W_ext, b_ext, ln_g, ln_b, W_cls):
    seq = np.asarray(seq, np.float32)
    starts = np.asarray(starts, np.int64)
    ends = np.asarray(ends, np.int64)
    mask = np.asarray(mention_mask, np.float32)

    # per-document entity selection matrix: ent = Sb^T @ seq[b]
    S_b = np.zeros((B, L, E), np.float32)
    denom = np.maximum(mask.sum(axis=2), 1.0)          # [B, E]
    w = mask * 0.5 / denom[:, :, None]                 # [B, E, M]
    for b in range(B):
        for e in range(E):
            np.add.at(S_b[b, :, e], starts[b, e] + 1, w[b, e])
            np.add.at(S_b[b, :, e], ends[b, e], w[b, e])

    cls_col = np.zeros((L, 1), np.float32)
    cls_col[0, 0] = 1.0

    shared = {
        "Wh": np.ascontiguousarray(np.asarray(W_head, np.float32).astype(np.float16)),
        "Wt": np.ascontiguousarray(np.asarray(W_tail, np.float32).astype(np.float16)),
        "bh": np.ascontiguousarray(np.asarray(b_head, np.float32).reshape(CT, 128).T),
        "bt": np.ascontiguousarray(np.asarray(b_tail, np.float32).reshape(CT, 128).T),
        # partition p = di*16+pj, kt = (g, ib2, jb); row k = g*4096 +
        # (ib2*8+di)*64 + jb*16 + pj
        "Wx": np.ascontiguousarray(
            np.asarray(W_ext).astype(np.float16)
            .reshape(G, 8, 8, 4, 16, EMB).transpose(2, 4, 0, 1, 3, 5)
            .reshape(128, KT * EMB)),
        "bx": np.ascontiguousarray(np.broadcast_to(np.asarray(b_ext, np.float32), (128, EMB))),
        "lng": np.ascontiguousarray(np.broadcast_to(np.asarray(ln_g, np.float32), (128, EMB))),
        "lnb": np.ascontiguousarray(np.broadcast_to(np.asarray(ln_b, np.float32), (128, EMB))),
        "Wc": np.ascontiguousarray(np.asarray(W_cls, np.float32).astype(np.float16)),
    }
    in_maps = []
    for core in range(N_CORES):
        b, ib = core // 4, core % 4
        S_core = np.concatenate(
            [S_b[b][:, ib * IB:(ib + 1) * IB], S_b[b], cls_col], axis=1)
        in_maps.append({
            "seq": np.ascontiguousarray(seq[b].astype(np.float16)),
            "S": np.ascontiguousarray(S_core.astype(np.float16)),
            **shared,
        })
    return in_maps


def kernel(**inputs) -> np.ndarray:
    nc = _get_module()
    in_maps = _build_inputs(**inputs)
    res = run_bass_kernel_spmd(nc, in_maps, core_ids=list(range(N_CORES)))
    outs = np.stack([res.results[c]["out"] for c in range(N_CORES)])  # [8,256,97]
    return outs.reshape(B, 4, IB, E, NL).reshape(B, E, E, NL)
